# revision 1
# baseline (speedup 1.0000x reference)
"""2-layer GCN (GCN-normalized adjacency, self-loops) on 8 TRN2 NeuronCores.

kernel(x, W1, W2, edge_index) -> [100000, 40] float32

Strategy (all hardcoded for N=100000, E arbitrary, IN=256, HID=64, OUT=40):
- Nodes partitioned contiguously across 8 cores (12544/core, padded 100352).
  Edges partitioned by destination owner; weights replicated.
- Per layer: local dense matmul emits the dinv-scaled feature table,
  AllGather replicates it (HBM), each core then gathers source rows with
  dma_gather (int16 indices -> 4 source windows of 25088 rows) and
  segment-sums per 128-dest tile via a one-hot matmul on the PE
  (one-hot built on DVE by comparing dest-local ids against an iota row).
- D^-1/2 (A+I) D^-1/2 folds into per-partition ACT copy scales: table rows
  carry dinv_src; psum epilogues carry dinv_dest (layer1: relu(psum) ->
  W2 matmul -> scale dinv^2; layer2: one-hot-stationary matmul so dests land
  on psum partitions -> scale dinv).
"""

import sys

sys.path.insert(0, "/opt/trn_rl_repo")

import hashlib
from contextlib import ExitStack
from dataclasses import dataclass

import numpy as np

F32 = None  # set on first use (lazy concourse import)
I16 = None


@dataclass(frozen=True)
class _Cfg:
    n: int = 100000
    n_cores: int = 8
    per: int = 12544
    windows: int = 4
    in_c: int = 256
    hid: int = 64
    out_f: int = 40
    out_pad: int = 64
    sg: int = 6
    call_max: int = 6144

    @property
    def n_pad(self):
        return self.per * self.n_cores

    @property
    def tiles(self):
        return self.per // 128

    @property
    def win_rows(self):
        return self.n_pad // self.windows


CFG = _Cfg()


def _round128(x):
    return (x + 127) // 128 * 128


def _preprocess(cfg, x, W1, W2, edge_index):
    n, per, T, W = cfg.n, cfg.per, cfg.tiles, cfg.windows
    ei = np.asarray(edge_index)
    dest = ei[0].astype(np.int64)
    src = ei[1].astype(np.int64)
    deg = np.bincount(dest, minlength=n).astype(np.float32) + 1.0
    dinv = (1.0 / np.sqrt(deg)).astype(np.float32)

    loops = np.arange(n, dtype=np.int64)
    dest = np.concatenate([dest, loops])
    src = np.concatenate([src, loops])

    core = dest // per
    t_loc = (dest % per) // 128
    w = src // cfg.win_rows
    key = (core * T + t_loc) * W + w
    order = np.lexsort((src, key))
    key_s = key[order]
    dest_s = dest[order]
    src_s = src[order]

    counts = np.bincount(key, minlength=cfg.n_cores * T * W).reshape(cfg.n_cores, T, W)
    caps = np.vectorize(_round128)(counts.max(axis=0))  # [T, W]
    tot = int(caps.sum())
    totch = tot // 128

    block_off = np.zeros((T, W), dtype=np.int64)
    block_off.reshape(-1)[1:] = np.cumsum(caps.reshape(-1))[:-1]

    group_start = np.zeros(cfg.n_cores * T * W, dtype=np.int64)
    group_start[1:] = np.cumsum(counts.reshape(-1))[:-1]
    rank = np.arange(dest.shape[0]) - group_start[key_s]

    idx_tm = np.zeros((cfg.n_cores, tot), dtype=np.int16)
    dloc_tm = np.full((cfg.n_cores, tot), 255.0, dtype=np.float32)
    core_s = key_s // (T * W)
    tw_s = key_s % (T * W)
    slot = block_off.reshape(-1)[tw_s] + rank
    idx_tm[core_s, slot] = (src_s - (src_s // cfg.win_rows) * cfg.win_rows).astype(
        np.int16
    )
    dloc_tm[core_s, slot] = (dest_s % 128).astype(np.float32)

    # gather-call layout: per supergroup x window, split to <= call_max slots
    n_sg = (T + cfg.sg - 1) // cfg.sg
    calls = []
    call_perm = np.empty(tot, dtype=np.int64)
    pos = 0
    for sgi in range(n_sg):
        ts_ = list(range(sgi * cfg.sg, min((sgi + 1) * cfg.sg, T)))
        for wi in range(W):
            cur = dict(sg=sgi, w=wi, n=0, tiles=[], boffs=[])
            for t in ts_:
                c = int(caps[t, wi])
                if c == 0:
                    continue
                if cur["n"] + c > cfg.call_max and cur["n"] > 0:
                    calls.append(cur)
                    cur = dict(sg=sgi, w=wi, n=0, tiles=[], boffs=[])
                call_perm[pos : pos + c] = np.arange(
                    block_off[t, wi], block_off[t, wi] + c
                )
                cur["tiles"].append(t)
                cur["boffs"].append(cur["n"])
                cur["n"] += c
                pos += c
            if cur["n"] > 0:
                calls.append(cur)
    assert pos == tot

    idx_call = idx_tm[:, call_perm]

    def wrap_idx(a):
        return np.tile(a.reshape(-1, 16).T, (8, 1))

    iota_mat = np.tile(np.arange(128, dtype=np.float32), (128, 1))
    W1T = np.ascontiguousarray(np.asarray(W1, np.float32).T)
    kc = cfg.in_c // 128
    W1T_pack = np.ascontiguousarray(W1T.reshape(kc, 128, cfg.hid).transpose(1, 0, 2))
    W2T_pad = np.zeros((cfg.hid, cfg.out_pad), dtype=np.float32)
    W2T_pad[:, : cfg.out_f] = np.asarray(W2, np.float32).T

    x = np.asarray(x, dtype=np.float32)
    in_maps = []
    for c in range(cfg.n_cores):
        lo, hi = c * per, min((c + 1) * per, n)
        cnt = max(hi - lo, 0)
        xT = np.zeros((cfg.in_c, per), dtype=np.float32)
        if cnt > 0:
            xT[:, :cnt] = x[lo : lo + cnt].T
        dv = np.zeros(per, dtype=np.float32)
        if cnt > 0:
            dv[:cnt] = dinv[lo : lo + cnt]
        in_maps.append(
            {
                "xT": xT,
                "W1T": W1T_pack,
                "W2T": W2T_pad,
                "dinv_col": np.ascontiguousarray(dv.reshape(T, 128).T),
                "dinv2_col": np.ascontiguousarray((dv * dv).reshape(T, 128).T),
                "iota": iota_mat,
                "idx": wrap_idx(idx_call[c]),
                "dloc": np.ascontiguousarray(dloc_tm[c].reshape(totch, 128).T),
            }
        )

    layout = dict(caps=caps, calls=calls, tot=tot, totch=totch)
    return in_maps, layout


def _build_kernel(cfg, layout):
    import concourse.mybir as mybir
    import concourse.tile as tile
    from concourse import bacc
    from concourse.library_config import mlp
    from concourse.tile import add_dep_helper

    F32 = mybir.dt.float32
    I16 = mybir.dt.int16

    caps = layout["caps"]
    calls = layout["calls"]
    totch = layout["totch"]
    tot = layout["tot"]
    T, W = cfg.tiles, cfg.windows
    kc = cfg.in_c // 128
    n_sg = (T + cfg.sg - 1) // cfg.sg
    maxcall_ch = max(c["n"] for c in calls) // 128

    nch = caps // 128
    tile_ch0 = np.zeros(T, dtype=np.int64)
    tile_ch0[1:] = np.cumsum(nch.sum(axis=1))[:-1]

    nc = bacc.Bacc("TRN2", num_swdge_queues=4)

    xT = nc.dram_tensor("xT", [cfg.in_c, cfg.per], F32, kind="ExternalInput")
    W1T = nc.dram_tensor("W1T", [128, kc, cfg.hid], F32, kind="ExternalInput")
    W2T = nc.dram_tensor("W2T", [cfg.hid, cfg.out_pad], F32, kind="ExternalInput")
    dinv_col = nc.dram_tensor("dinv_col", [128, T], F32, kind="ExternalInput")
    dinv2_col = nc.dram_tensor("dinv2_col", [128, T], F32, kind="ExternalInput")
    iota = nc.dram_tensor("iota", [128, 128], F32, kind="ExternalInput")
    idx_hbm = nc.dram_tensor("idx", [128, tot // 16], I16, kind="ExternalInput")
    dloc_hbm = nc.dram_tensor("dloc", [128, totch], F32, kind="ExternalInput")
    out_rows = nc.dram_tensor(
        "out_rows", [cfg.per, cfg.out_f], F32, kind="ExternalOutput"
    )
    table1_full = nc.dram_tensor(
        "table1_full", [cfg.n_pad, cfg.hid], F32, addr_space="Shared"
    )
    table2_full = nc.dram_tensor(
        "table2_full", [cfg.n_pad, cfg.out_pad], F32, addr_space="Shared"
    )

    rg = [list(range(cfg.n_cores))]
    qn = [0]

    with tile.TileContext(nc) as tc, ExitStack() as ctx:
        consts = ctx.enter_context(tc.tile_pool(name="consts", bufs=1))
        dram = ctx.enter_context(tc.tile_pool(name="dram", bufs=1, space="DRAM"))
        xpool = ctx.enter_context(tc.tile_pool(name="xpool", bufs=3))
        hpool = ctx.enter_context(tc.tile_pool(name="hpool", bufs=3))
        gpool = ctx.enter_context(tc.tile_pool(name="gpool", bufs=8))
        ipool = ctx.enter_context(tc.tile_pool(name="ipool", bufs=10))
        opool = ctx.enter_context(tc.tile_pool(name="opool", bufs=4))
        rpool = ctx.enter_context(tc.tile_pool(name="rpool", bufs=3))
        psum1 = ctx.enter_context(tc.tile_pool(name="psum1", bufs=3, space="PSUM"))
        psum2 = ctx.enter_context(tc.tile_pool(name="psum2", bufs=3, space="PSUM"))

        table1_local = dram.tile([cfg.per, cfg.hid], F32)
        table2_local = dram.tile([cfg.per, cfg.out_pad], F32)

        w1t = consts.tile([128, kc, cfg.hid], F32)
        nc.sync.dma_start(w1t[:], W1T[:])
        w2t = consts.tile([cfg.hid, cfg.out_pad], F32)
        nc.sync.dma_start(w2t[:], W2T[:])
        dvc = consts.tile([128, T], F32)
        nc.sync.dma_start(dvc[:], dinv_col[:])
        dv2c = consts.tile([128, T], F32)
        nc.sync.dma_start(dv2c[:], dinv2_col[:])
        iot = consts.tile([128, 1, 128], F32)
        nc.sync.dma_start(iot[:], iota[:, None, :])
        dlc = consts.tile([128, totch], F32)
        nc.sync.dma_start(dlc[:], dloc_hbm[:])

        lib_inst = nc.gpsimd.load_library(mlp)

        # phase 1: table1 = dinv * (x @ W1^T)
        XG = 8
        for g0 in range(0, T, XG):
            gn = min(XG, T - g0)
            xts = []
            for k in range(kc):
                xt = xpool.tile([128, XG * 128], F32, tag="xt", name=f"xt{g0}_{k}")
                nc.sync.dma_start(
                    xt[:, : gn * 128],
                    xT[k * 128 : (k + 1) * 128, g0 * 128 : (g0 + gn) * 128],
                )
                xts.append(xt)
            h1g = hpool.tile([128, XG, cfg.hid], F32, tag="h1g", name=f"h1g{g0}")
            for j in range(gn):
                t = g0 + j
                ph = psum2.tile([128, cfg.hid], F32, tag="ph", name=f"ph{t}")
                for k in range(kc):
                    nc.tensor.matmul(
                        ph[:],
                        lhsT=xts[k][:, j * 128 : (j + 1) * 128],
                        rhs=w1t[:, k, :],
                        start=(k == 0),
                        stop=(k == kc - 1),
                    )
                nc.scalar.activation(
                    h1g[:, j, :],
                    ph[:],
                    mybir.ActivationFunctionType.Copy,
                    scale=dvc[:, t : t + 1],
                )
            nc.sync.dma_start(
                table1_local[g0 * 128 : (g0 + gn) * 128, :].rearrange(
                    "(j p) f -> p j f", p=128
                ),
                h1g[:, :gn, :],
            )

        cc1 = nc.gpsimd.collective_compute(
            "AllGather",
            mybir.AluOpType.bypass,
            replica_groups=rg,
            ins=[table1_local.opt()],
            outs=[table1_full[:]],
        )
        cc2 = None

        def spmm(which, table_full, out_cb):
            feat = cfg.hid if which == 1 else cfg.out_pad
            cc = cc1 if which == 1 else cc2
            call_pos = 0
            for sgi in range(n_sg):
                ts_ = list(range(sgi * cfg.sg, min((sgi + 1) * cfg.sg, T)))
                gmap = {}
                for c in [c for c in calls if c["sg"] == sgi]:
                    n_slots = c["n"]
                    isb = ipool.tile(
                        [128, maxcall_ch * 8], I16, tag="idx",
                        name=f"i{which}_{sgi}_{c['w']}_{c['tiles'][0]}",
                    )
                    nc.sync.dma_start(
                        isb[:, : n_slots // 16],
                        idx_hbm[:, call_pos // 16 : (call_pos + n_slots) // 16],
                    )
                    gd = gpool.tile(
                        [128, maxcall_ch, feat], F32, tag="gd",
                        name=f"g{which}_{sgi}_{c['w']}_{c['tiles'][0]}",
                    )
                    win = table_full[
                        c["w"] * cfg.win_rows : (c["w"] + 1) * cfg.win_rows, :
                    ]
                    g = nc.gpsimd.dma_gather(
                        gd[:, : n_slots // 128, :],
                        win,
                        isb[:, : n_slots // 16],
                        n_slots,
                        n_slots,
                        feat,
                        single_packet=(n_slots <= 1024),
                        queue_num=qn[0] % 4,
                    )
                    qn[0] += 1
                    add_dep_helper(g.ins, cc.ins, reason="gather after allgather")
                    add_dep_helper(g.ins, lib_inst.ins, reason="gather after lib")
                    for t_, bo_ in zip(c["tiles"], c["boffs"]):
                        gmap[(t_, c["w"])] = (gd, bo_ // 128)
                    call_pos += n_slots
                for t in ts_:
                    nch_t = int(nch[t].sum())
                    if nch_t == 0:
                        continue
                    oh = opool.tile(
                        [128, nch_t, 128], F32, tag="oh", name=f"oh{which}_{t}"
                    )
                    ch0 = int(tile_ch0[t])
                    nc.vector.tensor_tensor(
                        out=oh[:],
                        in0=dlc[:, ch0 : ch0 + nch_t].to_broadcast([128, nch_t, 128]),
                        in1=iot[:, :, :].to_broadcast([128, nch_t, 128]),
                        op=mybir.AluOpType.is_equal,
                    )
                    if which == 1:
                        ps = psum1.tile([cfg.hid, 128], F32, tag="ps1", name=f"s{t}")
                    else:
                        ps = psum2.tile(
                            [128, cfg.out_pad], F32, tag="ph", name=f"s2_{t}"
                        )
                    ci = 0
                    for wi in range(W):
                        if nch[t, wi] == 0:
                            continue
                        gd, bo = gmap[(t, wi)]
                        for j in range(int(nch[t, wi])):
                            gsl = gd[:, bo + j, :]
                            osl = oh[:, ci, :]
                            if which == 1:
                                nc.tensor.matmul(
                                    ps[:], lhsT=gsl, rhs=osl,
                                    start=(ci == 0), stop=(ci == nch_t - 1),
                                )
                            else:
                                nc.tensor.matmul(
                                    ps[:], lhsT=osl, rhs=gsl,
                                    start=(ci == 0), stop=(ci == nch_t - 1),
                                )
                            ci += 1
                    out_cb(t, ps)

        h2gs = {}

        def spmm1_out(t, ps):
            rt = rpool.tile([cfg.hid, 128], F32, tag="rt", name=f"rt{t}")
            nc.scalar.activation(rt[:], ps[:], mybir.ActivationFunctionType.Relu)
            p2 = psum2.tile([128, cfg.out_pad], F32, tag="ph", name=f"p2_{t}")
            nc.tensor.matmul(p2[:], lhsT=rt[:], rhs=w2t[:], start=True, stop=True)
            sgi, j = t // cfg.sg, t % cfg.sg
            if sgi not in h2gs:
                h2gs[sgi] = hpool.tile(
                    [128, cfg.sg, cfg.out_pad], F32, tag="h2g", name=f"h2g{sgi}"
                )
            nc.scalar.activation(
                h2gs[sgi][:, j, :],
                p2[:],
                mybir.ActivationFunctionType.Copy,
                scale=dv2c[:, t : t + 1],
            )
            gn = min(cfg.sg, T - sgi * cfg.sg)
            if j == gn - 1:
                nc.sync.dma_start(
                    table2_local[
                        sgi * cfg.sg * 128 : (sgi * cfg.sg + gn) * 128, :
                    ].rearrange("(j p) f -> p j f", p=128),
                    h2gs[sgi][:, :gn, :],
                )

        spmm(1, table1_full, spmm1_out)

        cc2 = nc.gpsimd.collective_compute(
            "AllGather",
            mybir.AluOpType.bypass,
            replica_groups=rg,
            ins=[table2_local.opt()],
            outs=[table2_full[:]],
        )

        outgs = {}

        def spmm2_out(t, ps):
            sgi, j = t // cfg.sg, t % cfg.sg
            if sgi not in outgs:
                outgs[sgi] = hpool.tile(
                    [128, cfg.sg, cfg.out_pad], F32, tag="og", name=f"og{sgi}"
                )
            nc.scalar.activation(
                outgs[sgi][:, j, :],
                ps[:],
                mybir.ActivationFunctionType.Copy,
                scale=dvc[:, t : t + 1],
            )
            gn = min(cfg.sg, T - sgi * cfg.sg)
            if j == gn - 1:
                nc.sync.dma_start(
                    out_rows[
                        sgi * cfg.sg * 128 : (sgi * cfg.sg + gn) * 128, :
                    ].rearrange("(j p) f -> p j f", p=128),
                    outgs[sgi][:, :gn, : cfg.out_f],
                )

        spmm(2, table2_full, spmm2_out)

    nc.compile()
    return nc


class _Runner:
    """Cached PJRT runner (jit built once per compiled kernel)."""

    def __init__(self, nc, n_cores):
        import jax
        import numpy as np
        from jax.sharding import Mesh, PartitionSpec
        from jax.experimental.shard_map import shard_map
        import concourse.mybir as mybir
        from concourse.bass2jax import (
            _bass_exec_p,
            install_neuronx_cc_hook,
            partition_id_tensor,
        )

        install_neuronx_cc_hook()
        self.n_cores = n_cores
        partition_name = (
            nc.partition_id_tensor.name if nc.partition_id_tensor else None
        )
        in_names, out_names, out_avals, zero_outs = [], [], [], []
        import concourse.mybir as mb

        for alloc in nc.m.functions[0].allocations:
            if not isinstance(alloc, mb.MemoryLocationSet):
                continue
            name = alloc.memorylocations[0].name
            if alloc.kind == "ExternalInput":
                if name != partition_name:
                    in_names.append(name)
            elif alloc.kind == "ExternalOutput":
                out_names.append(name)
                shape = tuple(alloc.tensor_shape)
                dtype = mb.dt.np(alloc.dtype)
                out_avals.append(jax.core.ShapedArray(shape, dtype))
                zero_outs.append(np.zeros(shape, dtype))
        self.in_names = in_names
        self.out_names = out_names
        self.out_avals = out_avals
        self.zero_outs = zero_outs
        n_params = len(in_names)
        self.n_params = n_params
        all_in_names = in_names + out_names
        if partition_name is not None:
            all_in_names.append(partition_name)

        def _body(*args):
            operands = list(args)
            if partition_name is not None:
                operands.append(partition_id_tensor())
            outs = _bass_exec_p.bind(
                *operands,
                out_avals=tuple(out_avals),
                in_names=tuple(all_in_names),
                out_names=tuple(out_names),
                lowering_input_output_aliases=(),
                sim_require_finite=True,
                sim_require_nnan=True,
                nc=nc,
            )
            return tuple(outs)

        devices = jax.devices()[:n_cores]
        assert len(devices) >= n_cores, f"need {n_cores} cores, have {devices}"
        mesh = Mesh(np.asarray(devices[:n_cores]), ("core",))
        in_specs = (PartitionSpec("core"),) * (n_params + len(out_names))
        out_specs = (PartitionSpec("core"),) * len(out_names)
        self.fn = jax.jit(
            shard_map(
                _body,
                mesh=mesh,
                in_specs=in_specs,
                out_specs=out_specs,
                check_rep=False,
            ),
            keep_unused=True,
        )
        self._jax = jax

    def _concat(self, in_maps):
        per_core = [[np.asarray(m[name]) for name in self.in_names] for m in in_maps]
        concat_in = [
            np.concatenate([per_core[c][i] for c in range(self.n_cores)], axis=0)
            for i in range(self.n_params)
        ]
        concat_zeros = [
            np.zeros((self.n_cores * z.shape[0], *z.shape[1:]), z.dtype)
            for z in self.zero_outs
        ]
        return concat_in + concat_zeros

    def stage(self, in_maps):
        self._dev = self._jax.device_put(self._concat(in_maps))
        self._jax.block_until_ready(self._dev)

    def run_staged(self):
        outs = self.fn(*self._dev)
        self._jax.block_until_ready(outs)
        return outs

    def split(self, outs):
        return [
            {
                name: np.asarray(outs[i]).reshape(
                    self.n_cores, *self.out_avals[i].shape
                )[c]
                for i, name in enumerate(self.out_names)
            }
            for c in range(self.n_cores)
        ]

    def run(self, in_maps):
        outs = self.fn(*self._concat(in_maps))
        self._jax.block_until_ready(outs)
        return self.split(outs)


_CACHE = {}


def _get_runner(layout):
    caps_key = hashlib.sha256(np.ascontiguousarray(layout["caps"]).tobytes()).hexdigest()
    if caps_key not in _CACHE:
        nc = _build_kernel(CFG, layout)
        _CACHE[caps_key] = _Runner(nc, CFG.n_cores)
    return _CACHE[caps_key]


def kernel(x, W1, W2, edge_index):
    cfg = CFG
    in_maps, layout = _preprocess(cfg, x, W1, W2, edge_index)
    runner = _get_runner(layout)

    def _once():
        results = runner.run(in_maps)
        parts = [results[c]["out_rows"] for c in range(cfg.n_cores)]
        return np.ascontiguousarray(
            np.concatenate(parts, axis=0)[: cfg.n, :], dtype=np.float32
        )

    out = _once()
    # Flaky axon devices occasionally come up desynced and return garbage on
    # the first execution after a wedge; one cheap re-run is reliable.
    if not np.isfinite(out).all() or float(np.abs(out).sum()) == 0.0:
        out = _once()
    return out


# expose internals for the test harness
def _internals():
    return CFG, _preprocess, _build_kernel, _Runner



# revision 8
# speedup vs baseline: 4.6226x; 4.6226x over previous
"""2-layer GCN (GCN-normalized adjacency, self-loops) on 8 TRN2 NeuronCores.

kernel(x, W1, W2, edge_index) -> [100000, 40] float32

Strategy (all hardcoded for N=100000, E arbitrary, IN=256, HID=64, OUT=40):
- Nodes partitioned contiguously across 8 cores (12544/core, padded 100352).
  Edges partitioned by destination owner; weights replicated.
- Per layer: local dense matmul emits the dinv-scaled feature table,
  AllGather replicates it (HBM), each core then gathers source rows with
  dma_gather (int16 indices -> 4 source windows of 25088 rows) and
  segment-sums per 128-dest tile via a one-hot matmul on the PE
  (one-hot built on DVE by comparing dest-local ids against an iota row).
- D^-1/2 (A+I) D^-1/2 folds into per-partition ACT copy scales: table rows
  carry dinv_src; psum epilogues carry dinv_dest (layer1: relu(psum) ->
  W2 matmul -> scale dinv^2; layer2: one-hot-stationary matmul so dests land
  on psum partitions -> scale dinv).
"""

import sys

sys.path.insert(0, "/opt/trn_rl_repo")

import hashlib
from contextlib import ExitStack
from dataclasses import dataclass

import numpy as np

F32 = None  # set on first use (lazy concourse import)
I16 = None


@dataclass(frozen=True)
class _Cfg:
    n: int = 100000
    n_cores: int = 8
    per: int = 12544
    windows: int = 4
    in_c: int = 256
    hid: int = 64
    out_f: int = 40
    out_pad: int = 64
    sg: int = 6
    call_max: int = 6144

    @property
    def n_pad(self):
        return self.per * self.n_cores

    @property
    def tiles(self):
        return self.per // 128

    @property
    def win_rows(self):
        return self.n_pad // self.windows


CFG = _Cfg()


def _round128(x):
    return (x + 127) // 128 * 128


def _preprocess(cfg, x, W1, W2, edge_index):
    n, per, T, W = cfg.n, cfg.per, cfg.tiles, cfg.windows
    ei = np.asarray(edge_index)
    dest = ei[0].astype(np.int64)
    src = ei[1].astype(np.int64)
    deg = np.bincount(dest, minlength=n).astype(np.float32) + 1.0
    dinv = (1.0 / np.sqrt(deg)).astype(np.float32)

    loops = np.arange(n, dtype=np.int64)
    dest = np.concatenate([dest, loops])
    src = np.concatenate([src, loops])

    core = dest // per
    t_loc = (dest % per) // 128
    w = src // cfg.win_rows
    key = (core * T + t_loc) * W + w
    order = np.lexsort((src, key))
    key_s = key[order]
    dest_s = dest[order]
    src_s = src[order]

    counts = np.bincount(key, minlength=cfg.n_cores * T * W).reshape(cfg.n_cores, T, W)
    caps = np.vectorize(_round128)(counts.max(axis=0))  # [T, W]
    tot = int(caps.sum())
    totch = tot // 128

    block_off = np.zeros((T, W), dtype=np.int64)
    block_off.reshape(-1)[1:] = np.cumsum(caps.reshape(-1))[:-1]

    group_start = np.zeros(cfg.n_cores * T * W, dtype=np.int64)
    group_start[1:] = np.cumsum(counts.reshape(-1))[:-1]
    rank = np.arange(dest.shape[0]) - group_start[key_s]

    idx_tm = np.zeros((cfg.n_cores, tot), dtype=np.int16)
    dloc_tm = np.full((cfg.n_cores, tot), 255.0, dtype=np.float32)
    core_s = key_s // (T * W)
    tw_s = key_s % (T * W)
    slot = block_off.reshape(-1)[tw_s] + rank
    idx_tm[core_s, slot] = (src_s - (src_s // cfg.win_rows) * cfg.win_rows).astype(
        np.int16
    )
    dloc_tm[core_s, slot] = (dest_s % 128).astype(np.float32)

    # gather-call layout: per supergroup x window, split to <= call_max slots
    n_sg = (T + cfg.sg - 1) // cfg.sg
    calls = []
    call_perm = np.empty(tot, dtype=np.int64)
    pos = 0
    for sgi in range(n_sg):
        ts_ = list(range(sgi * cfg.sg, min((sgi + 1) * cfg.sg, T)))
        for wi in range(W):
            cur = dict(sg=sgi, w=wi, n=0, tiles=[], boffs=[])
            for t in ts_:
                c = int(caps[t, wi])
                if c == 0:
                    continue
                if cur["n"] + c > cfg.call_max and cur["n"] > 0:
                    calls.append(cur)
                    cur = dict(sg=sgi, w=wi, n=0, tiles=[], boffs=[])
                call_perm[pos : pos + c] = np.arange(
                    block_off[t, wi], block_off[t, wi] + c
                )
                cur["tiles"].append(t)
                cur["boffs"].append(cur["n"])
                cur["n"] += c
                pos += c
            if cur["n"] > 0:
                calls.append(cur)
    assert pos == tot

    idx_call = idx_tm[:, call_perm]

    def wrap_idx(a):
        return np.tile(a.reshape(-1, 16).T, (8, 1))

    iota_mat = np.tile(np.arange(128, dtype=np.float32), (128, 1))
    W1T = np.ascontiguousarray(np.asarray(W1, np.float32).T)
    kc = cfg.in_c // 128
    W1T_pack = np.ascontiguousarray(W1T.reshape(kc, 128, cfg.hid).transpose(1, 0, 2))
    W2T_pad = np.zeros((cfg.hid, cfg.out_pad), dtype=np.float32)
    W2T_pad[:, : cfg.out_f] = np.asarray(W2, np.float32).T

    x = np.asarray(x, dtype=np.float32)
    in_maps = []
    for c in range(cfg.n_cores):
        lo, hi = c * per, min((c + 1) * per, n)
        cnt = max(hi - lo, 0)
        xT = np.zeros((cfg.in_c, per), dtype=np.float32)
        if cnt > 0:
            xT[:, :cnt] = x[lo : lo + cnt].T
        dv = np.zeros(per, dtype=np.float32)
        if cnt > 0:
            dv[:cnt] = dinv[lo : lo + cnt]
        in_maps.append(
            {
                "xT": xT,
                "W1T": W1T_pack,
                "W2T": W2T_pad,
                "dinv_col": np.ascontiguousarray(dv.reshape(T, 128).T),
                "dinv2_col": np.ascontiguousarray((dv * dv).reshape(T, 128).T),
                "iota": iota_mat,
                "idx": wrap_idx(idx_call[c]),
                "dloc": np.ascontiguousarray(dloc_tm[c].reshape(totch, 128).T),
            }
        )

    layout = dict(caps=caps, calls=calls, tot=tot, totch=totch)
    return in_maps, layout


def _build_kernel(cfg, layout, sim=False, mode=None):
    # mode: "full" (real), "noag" (AllGather -> small local copy),
    # "nogather" (dma_gather -> contiguous stream of same bytes),
    # "spmm1chunk" (one matmul chunk per dest tile instead of nch_t).
    # Only "full" is correct; others are timing-ablation variants.
    mode = mode or ("noag" if sim else "full")
    sim = mode == "noag"
    import concourse.mybir as mybir
    import concourse.tile as tile
    from concourse import bacc
    from concourse.library_config import mlp
    from concourse.tile import add_dep_helper

    F32 = mybir.dt.float32
    I16 = mybir.dt.int16

    caps = layout["caps"]
    calls = layout["calls"]
    totch = layout["totch"]
    tot = layout["tot"]
    T, W = cfg.tiles, cfg.windows
    kc = cfg.in_c // 128
    n_sg = (T + cfg.sg - 1) // cfg.sg
    maxcall_ch = max(c["n"] for c in calls) // 128

    nch = caps // 128
    tile_ch0 = np.zeros(T, dtype=np.int64)
    tile_ch0[1:] = np.cumsum(nch.sum(axis=1))[:-1]

    nc = bacc.Bacc("TRN2", num_swdge_queues=4)

    xT = nc.dram_tensor("xT", [cfg.in_c, cfg.per], F32, kind="ExternalInput")
    W1T = nc.dram_tensor("W1T", [128, kc, cfg.hid], F32, kind="ExternalInput")
    W2T = nc.dram_tensor("W2T", [cfg.hid, cfg.out_pad], F32, kind="ExternalInput")
    dinv_col = nc.dram_tensor("dinv_col", [128, T], F32, kind="ExternalInput")
    dinv2_col = nc.dram_tensor("dinv2_col", [128, T], F32, kind="ExternalInput")
    iota = nc.dram_tensor("iota", [128, 128], F32, kind="ExternalInput")
    idx_hbm = nc.dram_tensor("idx", [128, tot // 16], I16, kind="ExternalInput")
    dloc_hbm = nc.dram_tensor("dloc", [128, totch], F32, kind="ExternalInput")
    out_rows = nc.dram_tensor(
        "out_rows", [cfg.per, cfg.out_f], F32, kind="ExternalOutput"
    )
    table1_full = nc.dram_tensor(
        "table1_full", [cfg.n_pad, cfg.hid], F32, addr_space="Shared"
    )
    table2_full = nc.dram_tensor(
        "table2_full", [cfg.n_pad, cfg.out_pad], F32, addr_space="Shared"
    )

    rg = [list(range(cfg.n_cores))]
    qn = [0]

    with tile.TileContext(nc) as tc, ExitStack() as ctx:
        consts = ctx.enter_context(tc.tile_pool(name="consts", bufs=1))
        dram = ctx.enter_context(tc.tile_pool(name="dram", bufs=1, space="DRAM"))
        xpool = ctx.enter_context(tc.tile_pool(name="xpool", bufs=3))
        hpool = ctx.enter_context(tc.tile_pool(name="hpool", bufs=3))
        gpool = ctx.enter_context(tc.tile_pool(name="gpool", bufs=8))
        ipool = ctx.enter_context(tc.tile_pool(name="ipool", bufs=10))
        opool = ctx.enter_context(tc.tile_pool(name="opool", bufs=4))
        rpool = ctx.enter_context(tc.tile_pool(name="rpool", bufs=3))
        psum1 = ctx.enter_context(tc.tile_pool(name="psum1", bufs=3, space="PSUM"))
        psum2 = ctx.enter_context(tc.tile_pool(name="psum2", bufs=3, space="PSUM"))

        table1_local = dram.tile([cfg.per, cfg.hid], F32)
        table2_local = dram.tile([cfg.per, cfg.out_pad], F32)

        w1t = consts.tile([128, kc, cfg.hid], F32)
        nc.sync.dma_start(w1t[:], W1T[:])
        w2t = consts.tile([cfg.hid, cfg.out_pad], F32)
        nc.sync.dma_start(w2t[:], W2T[:])
        dvc = consts.tile([128, T], F32)
        nc.sync.dma_start(dvc[:], dinv_col[:])
        dv2c = consts.tile([128, T], F32)
        nc.sync.dma_start(dv2c[:], dinv2_col[:])
        iot = consts.tile([128, 1, 128], F32)
        nc.sync.dma_start(iot[:], iota[:, None, :])
        dlc = consts.tile([128, totch], F32)
        nc.sync.dma_start(dlc[:], dloc_hbm[:])

        lib_inst = nc.gpsimd.load_library(mlp)

        # phase 1: table1 = dinv * (x @ W1^T)
        XG = 8
        for g0 in range(0, T, XG):
            gn = min(XG, T - g0)
            xts = []
            for k in range(kc):
                xt = xpool.tile([128, XG * 128], F32, tag="xt", name=f"xt{g0}_{k}")
                nc.sync.dma_start(
                    xt[:, : gn * 128],
                    xT[k * 128 : (k + 1) * 128, g0 * 128 : (g0 + gn) * 128],
                )
                xts.append(xt)
            h1g = hpool.tile([128, XG, cfg.hid], F32, tag="h1g", name=f"h1g{g0}")
            for j in range(gn):
                t = g0 + j
                ph = psum2.tile([128, cfg.hid], F32, tag="ph", name=f"ph{t}")
                for k in range(kc):
                    nc.tensor.matmul(
                        ph[:],
                        lhsT=xts[k][:, j * 128 : (j + 1) * 128],
                        rhs=w1t[:, k, :],
                        start=(k == 0),
                        stop=(k == kc - 1),
                    )
                nc.scalar.activation(
                    h1g[:, j, :],
                    ph[:],
                    mybir.ActivationFunctionType.Copy,
                    scale=dvc[:, t : t + 1],
                )
            nc.sync.dma_start(
                table1_local[g0 * 128 : (g0 + gn) * 128, :].rearrange(
                    "(j p) f -> p j f", p=128
                ),
                h1g[:, :gn, :],
            )

        if sim:
            cc1 = nc.sync.dma_start(table1_full[: cfg.per, :], table1_local[:])
        else:
            cc1 = nc.gpsimd.collective_compute(
                "AllGather",
                mybir.AluOpType.bypass,
                replica_groups=rg,
                ins=[table1_local.opt()],
                outs=[table1_full[:]],
            )
        cc2 = None

        def spmm(which, table_full, out_cb):
            feat = cfg.hid if which == 1 else cfg.out_pad
            cc = cc1 if which == 1 else cc2
            call_pos = 0
            for sgi in range(n_sg):
                ts_ = list(range(sgi * cfg.sg, min((sgi + 1) * cfg.sg, T)))
                gmap = {}
                for c in [c for c in calls if c["sg"] == sgi]:
                    n_slots = c["n"]
                    isb = ipool.tile(
                        [128, maxcall_ch * 8], I16, tag="idx",
                        name=f"i{which}_{sgi}_{c['w']}_{c['tiles'][0]}",
                    )
                    nc.sync.dma_start(
                        isb[:, : n_slots // 16],
                        idx_hbm[:, call_pos // 16 : (call_pos + n_slots) // 16],
                    )
                    gd = gpool.tile(
                        [128, maxcall_ch, feat], F32, tag="gd",
                        name=f"g{which}_{sgi}_{c['w']}_{c['tiles'][0]}",
                    )
                    win = table_full[
                        c["w"] * cfg.win_rows : (c["w"] + 1) * cfg.win_rows, :
                    ]
                    if mode == "nogather":
                        g = nc.sync.dma_start(
                            gd[:, : n_slots // 128, :],
                            win[:n_slots, :].rearrange("(c p) f -> p c f", p=128),
                        )
                    else:
                        g = nc.gpsimd.dma_gather(
                            gd[:, : n_slots // 128, :],
                            win,
                            isb[:, : n_slots // 16],
                            n_slots,
                            n_slots,
                            feat,
                            single_packet=(n_slots <= 1024),
                            queue_num=qn[0] % 4,
                        )
                    qn[0] += 1
                    add_dep_helper(g.ins, cc.ins, reason="gather after allgather")
                    add_dep_helper(g.ins, lib_inst.ins, reason="gather after lib")
                    for t_, bo_ in zip(c["tiles"], c["boffs"]):
                        gmap[(t_, c["w"])] = (gd, bo_ // 128)
                    call_pos += n_slots
                for t in ts_:
                    nch_t = int(nch[t].sum())
                    if nch_t == 0:
                        continue
                    if mode == "spmm1chunk":
                        nch_t = 1
                    oh = opool.tile(
                        [128, nch_t, 128], F32, tag="oh", name=f"oh{which}_{t}"
                    )
                    ch0 = int(tile_ch0[t])
                    nc.vector.tensor_tensor(
                        out=oh[:],
                        in0=dlc[:, ch0 : ch0 + nch_t].to_broadcast([128, nch_t, 128]),
                        in1=iot[:, :, :].to_broadcast([128, nch_t, 128]),
                        op=mybir.AluOpType.is_equal,
                    )
                    if which == 1:
                        ps = psum1.tile([cfg.hid, 128], F32, tag="ps1", name=f"s{t}")
                    else:
                        ps = psum2.tile(
                            [128, cfg.out_pad], F32, tag="ph", name=f"s2_{t}"
                        )
                    ci = 0
                    for wi in range(W):
                        if nch[t, wi] == 0 or ci >= nch_t:
                            continue
                        gd, bo = gmap[(t, wi)]
                        for j in range(int(nch[t, wi])):
                            if ci >= nch_t:
                                break
                            gsl = gd[:, bo + j, :]
                            osl = oh[:, ci, :]
                            if which == 1:
                                nc.tensor.matmul(
                                    ps[:], lhsT=gsl, rhs=osl,
                                    start=(ci == 0), stop=(ci == nch_t - 1),
                                )
                            else:
                                nc.tensor.matmul(
                                    ps[:], lhsT=osl, rhs=gsl,
                                    start=(ci == 0), stop=(ci == nch_t - 1),
                                )
                            ci += 1
                    out_cb(t, ps)

        h2gs = {}

        def spmm1_out(t, ps):
            rt = rpool.tile([cfg.hid, 128], F32, tag="rt", name=f"rt{t}")
            nc.scalar.activation(rt[:], ps[:], mybir.ActivationFunctionType.Relu)
            p2 = psum2.tile([128, cfg.out_pad], F32, tag="ph", name=f"p2_{t}")
            nc.tensor.matmul(p2[:], lhsT=rt[:], rhs=w2t[:], start=True, stop=True)
            sgi, j = t // cfg.sg, t % cfg.sg
            if sgi not in h2gs:
                h2gs[sgi] = hpool.tile(
                    [128, cfg.sg, cfg.out_pad], F32, tag="h2g", name=f"h2g{sgi}"
                )
            nc.scalar.activation(
                h2gs[sgi][:, j, :],
                p2[:],
                mybir.ActivationFunctionType.Copy,
                scale=dv2c[:, t : t + 1],
            )
            gn = min(cfg.sg, T - sgi * cfg.sg)
            if j == gn - 1:
                nc.sync.dma_start(
                    table2_local[
                        sgi * cfg.sg * 128 : (sgi * cfg.sg + gn) * 128, :
                    ].rearrange("(j p) f -> p j f", p=128),
                    h2gs[sgi][:, :gn, :],
                )

        spmm(1, table1_full, spmm1_out)

        if sim:
            cc2 = nc.sync.dma_start(table2_full[: cfg.per, :], table2_local[:])
        else:
            cc2 = nc.gpsimd.collective_compute(
                "AllGather",
                mybir.AluOpType.bypass,
                replica_groups=rg,
                ins=[table2_local.opt()],
                outs=[table2_full[:]],
            )

        outgs = {}

        def spmm2_out(t, ps):
            sgi, j = t // cfg.sg, t % cfg.sg
            if sgi not in outgs:
                outgs[sgi] = hpool.tile(
                    [128, cfg.sg, cfg.out_pad], F32, tag="og", name=f"og{sgi}"
                )
            nc.scalar.activation(
                outgs[sgi][:, j, :],
                ps[:],
                mybir.ActivationFunctionType.Copy,
                scale=dvc[:, t : t + 1],
            )
            gn = min(cfg.sg, T - sgi * cfg.sg)
            if j == gn - 1:
                nc.sync.dma_start(
                    out_rows[
                        sgi * cfg.sg * 128 : (sgi * cfg.sg + gn) * 128, :
                    ].rearrange("(j p) f -> p j f", p=128),
                    outgs[sgi][:, :gn, : cfg.out_f],
                )

        spmm(2, table2_full, spmm2_out)

    nc.compile()
    return nc


class _Runner:
    """Cached PJRT runner (jit built once per compiled kernel)."""

    def __init__(self, nc, n_cores):
        import jax
        import numpy as np
        from jax.sharding import Mesh, PartitionSpec
        from jax.experimental.shard_map import shard_map
        import concourse.mybir as mybir
        from concourse.bass2jax import (
            _bass_exec_p,
            install_neuronx_cc_hook,
            partition_id_tensor,
        )

        install_neuronx_cc_hook()
        self.n_cores = n_cores
        partition_name = (
            nc.partition_id_tensor.name if nc.partition_id_tensor else None
        )
        in_names, out_names, out_avals, zero_outs = [], [], [], []
        import concourse.mybir as mb

        for alloc in nc.m.functions[0].allocations:
            if not isinstance(alloc, mb.MemoryLocationSet):
                continue
            name = alloc.memorylocations[0].name
            if alloc.kind == "ExternalInput":
                if name != partition_name:
                    in_names.append(name)
            elif alloc.kind == "ExternalOutput":
                out_names.append(name)
                shape = tuple(alloc.tensor_shape)
                dtype = mb.dt.np(alloc.dtype)
                out_avals.append(jax.core.ShapedArray(shape, dtype))
                zero_outs.append(np.zeros(shape, dtype))
        self.in_names = in_names
        self.out_names = out_names
        self.out_avals = out_avals
        self.zero_outs = zero_outs
        n_params = len(in_names)
        self.n_params = n_params
        all_in_names = in_names + out_names
        if partition_name is not None:
            all_in_names.append(partition_name)

        def _body(*args):
            operands = list(args)
            if partition_name is not None:
                operands.append(partition_id_tensor())
            outs = _bass_exec_p.bind(
                *operands,
                out_avals=tuple(out_avals),
                in_names=tuple(all_in_names),
                out_names=tuple(out_names),
                lowering_input_output_aliases=(),
                sim_require_finite=True,
                sim_require_nnan=True,
                nc=nc,
            )
            return tuple(outs)

        devices = jax.devices()[:n_cores]
        assert len(devices) >= n_cores, f"need {n_cores} cores, have {devices}"
        mesh = Mesh(np.asarray(devices[:n_cores]), ("core",))
        in_specs = (PartitionSpec("core"),) * (n_params + len(out_names))
        out_specs = (PartitionSpec("core"),) * len(out_names)
        self.fn = jax.jit(
            shard_map(
                _body,
                mesh=mesh,
                in_specs=in_specs,
                out_specs=out_specs,
                check_rep=False,
            ),
            keep_unused=True,
        )
        self._jax = jax

    def _concat(self, in_maps):
        per_core = [[np.asarray(m[name]) for name in self.in_names] for m in in_maps]
        concat_in = [
            np.concatenate([per_core[c][i] for c in range(self.n_cores)], axis=0)
            for i in range(self.n_params)
        ]
        concat_zeros = [
            np.zeros((self.n_cores * z.shape[0], *z.shape[1:]), z.dtype)
            for z in self.zero_outs
        ]
        return concat_in + concat_zeros

    def stage(self, in_maps):
        self._dev = self._jax.device_put(self._concat(in_maps))
        self._jax.block_until_ready(self._dev)

    def run_staged(self):
        outs = self.fn(*self._dev)
        self._jax.block_until_ready(outs)
        return outs

    def split(self, outs):
        return [
            {
                name: np.asarray(outs[i]).reshape(
                    self.n_cores, *self.out_avals[i].shape
                )[c]
                for i, name in enumerate(self.out_names)
            }
            for c in range(self.n_cores)
        ]

    def run(self, in_maps):
        outs = self.fn(*self._concat(in_maps))
        self._jax.block_until_ready(outs)
        return self.split(outs)


_CACHE = {}


def _get_runner(layout):
    caps_key = hashlib.sha256(np.ascontiguousarray(layout["caps"]).tobytes()).hexdigest()
    if caps_key not in _CACHE:
        nc = _build_kernel(CFG, layout)
        _CACHE[caps_key] = _Runner(nc, CFG.n_cores)
    return _CACHE[caps_key]


def kernel(x, W1, W2, edge_index):
    cfg = CFG
    in_maps, layout = _preprocess(cfg, x, W1, W2, edge_index)
    runner = _get_runner(layout)

    def _once():
        results = runner.run(in_maps)
        parts = [results[c]["out_rows"] for c in range(cfg.n_cores)]
        return np.ascontiguousarray(
            np.concatenate(parts, axis=0)[: cfg.n, :], dtype=np.float32
        )

    out = _once()
    # Flaky axon devices occasionally come up desynced and return garbage on
    # the first execution after a wedge; one cheap re-run is reliable.
    if not np.isfinite(out).all() or float(np.abs(out).sum()) == 0.0:
        out = _once()
    return out


# expose internals for the test harness
def _internals():
    return CFG, _preprocess, _build_kernel, _Runner



# revision 9
# speedup vs baseline: 8.3392x; 1.8040x over previous
"""2-layer GCN on 8 TRN2 NeuronCores — V2 (bf16 SpMM path).

kernel(x, W1, W2, edge_index) -> [100000, 40] float32

Changes vs V1:
- bf16 tables/one-hot/matmuls (PE 4x, DVE 2x); psum accumulation stays fp32.
- Tables stored [n, 128] bf16 (256B rows: dma_gather elem_size=128).
  Upper 64 cols are never written (garbage) and sliced off before the PE.
- Self-loops removed from the edge list; their contribution is added per
  dest tile with one identity-one-hot matmul sourcing the SBUF-resident
  local table tile (saves ~6% gather/PE/DVE).
"""

import sys

sys.path.insert(0, "/opt/trn_rl_repo")

import hashlib
from contextlib import ExitStack
from dataclasses import dataclass

import numpy as np
import ml_dtypes

BF16 = ml_dtypes.bfloat16


@dataclass(frozen=True)
class _Cfg:
    n: int = 100000
    n_cores: int = 8
    per: int = 12544
    windows: int = 4
    in_c: int = 256
    hid: int = 64
    out_f: int = 40
    out_pad: int = 64
    row_pad: int = 128  # table row width in bf16 elems (256B)
    sg: int = 7
    call_max: int = 1024
    dma_scratch: int = 16384
    single_packet: int = 1
    ag_chunks: int = 2  # split each AllGather into this many pipelined chunks

    @property
    def rows_per_chunk(self):
        return self.per // self.ag_chunks

    @property
    def n_pad(self):
        return self.per * self.n_cores

    @property
    def tiles(self):
        return self.per // 128

    @property
    def win_rows(self):
        return self.n_pad // self.windows


CFG = _Cfg()


def _round128(x):
    return (x + 127) // 128 * 128


def _preprocess(cfg, x, W1, W2, edge_index):
    n, per, T, W = cfg.n, cfg.per, cfg.tiles, cfg.windows
    ei = np.asarray(edge_index)
    dest = ei[0].astype(np.int64)
    src = ei[1].astype(np.int64)
    deg = np.bincount(dest, minlength=n).astype(np.float32) + 1.0
    dinv = (1.0 / np.sqrt(deg)).astype(np.float32)
    # self-loops handled by identity matmuls in-kernel (not in edge list)

    core = dest // per
    t_loc = (dest % per) // 128
    if cfg.ag_chunks > 1:
        # chunk-major full-table layout: chunk q holds every core's local rows
        # [q*rpc, (q+1)*rpc); within it rows are core-major. Windows subdivide
        # chunks so the gather idx stays int16.
        rpc = cfg.rows_per_chunk
        s_core = src // per
        s_r = src % per
        chunk_pos = s_core * rpc + (s_r % rpc)
        w = (W // cfg.ag_chunks) * (s_r // rpc) + chunk_pos // cfg.win_rows
        win_idx = chunk_pos % cfg.win_rows
    else:
        w = src // cfg.win_rows
        win_idx = src - (src // cfg.win_rows) * cfg.win_rows
    key = (core * T + t_loc) * W + w
    order = np.lexsort((src, key))
    key_s = key[order]
    dest_s = dest[order]
    src_s = src[order]

    counts = np.bincount(key, minlength=cfg.n_cores * T * W).reshape(cfg.n_cores, T, W)
    caps = np.vectorize(_round128)(counts.max(axis=0))  # [T, W]
    tot = int(caps.sum())
    totch = tot // 128

    block_off = np.zeros((T, W), dtype=np.int64)
    block_off.reshape(-1)[1:] = np.cumsum(caps.reshape(-1))[:-1]

    group_start = np.zeros(cfg.n_cores * T * W, dtype=np.int64)
    group_start[1:] = np.cumsum(counts.reshape(-1))[:-1]
    rank = np.arange(dest.shape[0]) - group_start[key_s]

    idx_tm = np.zeros((cfg.n_cores, tot), dtype=np.int16)
    dloc_tm = np.full((cfg.n_cores, tot), 255.0, dtype=np.float32)
    core_s = key_s // (T * W)
    tw_s = key_s % (T * W)
    slot = block_off.reshape(-1)[tw_s] + rank
    win_idx_s = win_idx[order]
    idx_tm[core_s, slot] = win_idx_s.astype(np.int16)
    dloc_tm[core_s, slot] = (dest_s % 128).astype(np.float32)

    # gather-call layout: chunk-granular, <= call_max slots per call so one
    # call never overflows the ~1024-descriptor SWDGE ring (which would stall
    # descriptor-gen on the Pool engine and serialize all queues).
    n_sg = (T + cfg.sg - 1) // cfg.sg
    calls = []
    call_perm = np.empty(tot, dtype=np.int64)
    pos = 0
    for sgi in range(n_sg):
        ts_ = list(range(sgi * cfg.sg, min((sgi + 1) * cfg.sg, T)))
        for wi in range(W):
            cur = dict(sg=sgi, w=wi, n=0, chunks=[])
            for t in ts_:
                c = int(caps[t, wi])
                for j in range(c // 128):
                    if cur["n"] + 128 > cfg.call_max:
                        calls.append(cur)
                        cur = dict(sg=sgi, w=wi, n=0, chunks=[])
                    call_perm[pos : pos + 128] = np.arange(
                        block_off[t, wi] + j * 128, block_off[t, wi] + (j + 1) * 128
                    )
                    cur["chunks"].append((t, cur["n"] // 128))
                    cur["n"] += 128
                    pos += 128
            if cur["n"] > 0:
                calls.append(cur)
    assert pos == tot

    idx_call = idx_tm[:, call_perm]
    # NOTE: do NOT mark trailing pad slots with idx=-1. The gather ucode trims
    # trailing negatives on the Q7 side but the decode stage reserves ring
    # space from the untrimmed register count; when the trim crosses a
    # 128-chunk boundary the DescriptorStream ANT_ASSERT fires and wedges the
    # exec unit. Pad slots gather row 0 (finite) and are killed by the zero
    # one-hot coefficient.

    def wrap_idx(a):
        return np.tile(a.reshape(-1, 16).T, (8, 1))

    iota_mat = np.tile(np.arange(128, dtype=np.float32), (128, 1)).astype(BF16)
    ident = np.eye(128, dtype=np.float32).astype(BF16)
    W1T = np.ascontiguousarray(np.asarray(W1, np.float32).T)
    kc = cfg.in_c // 128
    W1T_pack = np.ascontiguousarray(
        W1T.reshape(kc, 128, cfg.hid).transpose(1, 0, 2)
    ).astype(BF16)
    W2T_pad = np.zeros((cfg.hid, cfg.out_pad), dtype=np.float32)
    W2T_pad[:, : cfg.out_f] = np.asarray(W2, np.float32).T
    W2T_pad = W2T_pad.astype(BF16)

    x = np.asarray(x, dtype=np.float32)
    in_maps = []
    for c in range(cfg.n_cores):
        lo, hi = c * per, min((c + 1) * per, n)
        cnt = max(hi - lo, 0)
        xT = np.zeros((cfg.in_c, per), dtype=np.float32)
        if cnt > 0:
            xT[:, :cnt] = x[lo : lo + cnt].T
        dv = np.zeros(per, dtype=np.float32)
        if cnt > 0:
            dv[:cnt] = dinv[lo : lo + cnt]
        in_maps.append(
            {
                "xT": xT.astype(BF16),
                "W1T": W1T_pack,
                "W2T": W2T_pad,
                "dinv_col": np.ascontiguousarray(dv.reshape(T, 128).T),
                "dinv2_col": np.ascontiguousarray((dv * dv).reshape(T, 128).T),
                "iota": iota_mat,
                "ident": ident,
                "idx": wrap_idx(idx_call[c]),
                "dloc": np.ascontiguousarray(
                    dloc_tm[c].reshape(totch, 128).T
                ).astype(BF16),
            }
        )

    layout = dict(caps=caps, calls=calls, tot=tot, totch=totch)
    return in_maps, layout


def _build_kernel(cfg, layout, sim=False, mode=None):
    mode = mode or ("noag" if sim else "full")
    sim = mode == "noag"

    import concourse.mybir as mybir
    import concourse.tile as tile
    from concourse import bacc
    from concourse.library_config import mlp
    from concourse.tile import add_dep_helper

    F32 = mybir.dt.float32
    BF = mybir.dt.bfloat16
    I16 = mybir.dt.int16

    caps = layout["caps"]
    calls = layout["calls"]
    totch = layout["totch"]
    tot = layout["tot"]
    T, W = cfg.tiles, cfg.windows
    kc = cfg.in_c // 128
    n_sg = (T + cfg.sg - 1) // cfg.sg
    maxcall_ch = max(c["n"] for c in calls) // 128
    RP = cfg.row_pad

    nch = caps // 128
    tile_ch0 = np.zeros(T, dtype=np.int64)
    tile_ch0[1:] = np.cumsum(nch.sum(axis=1))[:-1]

    nc = bacc.Bacc(
        "TRN2", num_swdge_queues=4, dynamic_dma_scratch_size=cfg.dma_scratch
    )

    xT = nc.dram_tensor("xT", [cfg.in_c, cfg.per], BF, kind="ExternalInput")
    W1T = nc.dram_tensor("W1T", [128, kc, cfg.hid], BF, kind="ExternalInput")
    W2T = nc.dram_tensor("W2T", [cfg.hid, cfg.out_pad], BF, kind="ExternalInput")
    dinv_col = nc.dram_tensor("dinv_col", [128, T], F32, kind="ExternalInput")
    dinv2_col = nc.dram_tensor("dinv2_col", [128, T], F32, kind="ExternalInput")
    iota = nc.dram_tensor("iota", [128, 128], BF, kind="ExternalInput")
    ident_h = nc.dram_tensor("ident", [128, 128], BF, kind="ExternalInput")
    idx_hbm = nc.dram_tensor("idx", [128, tot // 16], I16, kind="ExternalInput")
    dloc_hbm = nc.dram_tensor("dloc", [128, totch], BF, kind="ExternalInput")
    out_rows = nc.dram_tensor(
        "out_rows", [cfg.per, cfg.out_f], F32, kind="ExternalOutput"
    )
    table1_full = nc.dram_tensor(
        "table1_full", [cfg.n_pad, RP], BF, addr_space="Shared"
    )
    table2_full = nc.dram_tensor(
        "table2_full", [cfg.n_pad, RP], BF, addr_space="Shared"
    )

    rg = [list(range(cfg.n_cores))]
    qn = [0]

    with tile.TileContext(nc) as tc, ExitStack() as ctx:
        consts = ctx.enter_context(tc.tile_pool(name="consts", bufs=1))
        dram = ctx.enter_context(tc.tile_pool(name="dram", bufs=1, space="DRAM"))
        xpool = ctx.enter_context(tc.tile_pool(name="xpool", bufs=3))
        keep = ctx.enter_context(tc.tile_pool(name="keep", bufs=1))
        gpool = ctx.enter_context(tc.tile_pool(name="gpool", bufs=16))
        opool = ctx.enter_context(tc.tile_pool(name="opool", bufs=4))
        rpool = ctx.enter_context(tc.tile_pool(name="rpool", bufs=3))
        hpool = ctx.enter_context(tc.tile_pool(name="hpool", bufs=3))
        psum1 = ctx.enter_context(tc.tile_pool(name="psum1", bufs=3, space="PSUM"))
        psum2 = ctx.enter_context(tc.tile_pool(name="psum2", bufs=3, space="PSUM"))

        Q = cfg.ag_chunks
        rpc = cfg.rows_per_chunk
        tiles_pc = T // Q  # tiles per AG chunk
        t1loc = [
            dram.tile([rpc, RP], BF, name=f"t1loc{q}") for q in range(Q)
        ]
        t2loc = [
            dram.tile([rpc, RP], BF, name=f"t2loc{q}") for q in range(Q)
        ]

        w1t = consts.tile([128, kc, cfg.hid], BF)
        nc.sync.dma_start(w1t[:], W1T[:])
        w2t = consts.tile([cfg.hid, cfg.out_pad], BF)
        nc.sync.dma_start(w2t[:], W2T[:])
        dvc = consts.tile([128, T], F32)
        nc.sync.dma_start(dvc[:], dinv_col[:])
        dv2c = consts.tile([128, T], F32)
        nc.sync.dma_start(dv2c[:], dinv2_col[:])
        iot = consts.tile([128, 1, 128], BF)
        nc.sync.dma_start(iot[:], iota[:, None, :])
        idn = consts.tile([128, 128], BF)
        nc.sync.dma_start(idn[:], ident_h[:])
        dlc = consts.tile([128, totch], BF)
        nc.sync.dma_start(dlc[:], dloc_hbm[:])
        isb_all = consts.tile([128, tot // 16], I16)
        nc.sync.dma_start(isb_all[:], idx_hbm[:])

        lib_inst = nc.gpsimd.load_library(mlp)

        # local table tiles stay in SBUF for self-loop matmuls + AG source
        h1full = keep.tile([128, T, cfg.hid], BF)
        t2full = keep.tile([128, T, cfg.out_pad], BF)

        # phase 1: table1 = dinv * (x @ W1^T)
        XG = cfg.sg
        for g0 in range(0, T, XG):
            gn = min(XG, T - g0)
            xts = []
            for k in range(kc):
                xt = xpool.tile([128, XG * 128], BF, tag="xt", name=f"xt{g0}_{k}")
                nc.sync.dma_start(
                    xt[:, : gn * 128],
                    xT[k * 128 : (k + 1) * 128, g0 * 128 : (g0 + gn) * 128],
                )
                xts.append(xt)
            for j in range(gn):
                t = g0 + j
                ph = psum2.tile([128, cfg.hid], F32, tag="ph", name=f"ph{t}")
                for k in range(kc):
                    nc.tensor.matmul(
                        ph[:],
                        lhsT=xts[k][:, j * 128 : (j + 1) * 128],
                        rhs=w1t[:, k, :],
                        start=(k == 0),
                        stop=(k == kc - 1),
                    )
                nc.scalar.activation(
                    h1full[:, t, :],
                    ph[:],
                    mybir.ActivationFunctionType.Copy,
                    scale=dvc[:, t : t + 1],
                )
            q = g0 // tiles_pc
            r0 = (g0 - q * tiles_pc) * 128
            nc.sync.dma_start(
                t1loc[q][r0 : r0 + gn * 128, : cfg.hid].rearrange(
                    "(j p) f -> p j f", p=128
                ),
                h1full[:, g0 : g0 + gn, :],
            )

        def make_ag(loc_tiles, table_full, feat, tag):
            ccs = []
            for q in range(Q):
                if sim:
                    cc = nc.sync.dma_start(
                        table_full[q * 8 * rpc : q * 8 * rpc + rpc, :feat],
                        loc_tiles[q][:, :feat],
                    )
                else:
                    cc = nc.gpsimd.collective_compute(
                        "AllGather",
                        mybir.AluOpType.bypass,
                        replica_groups=rg,
                        ins=[loc_tiles[q].opt()],
                        outs=[table_full[q * 8 * rpc : (q + 1) * 8 * rpc, :]],
                    )
                ccs.append(cc)
            return ccs

        cc1 = make_ag(t1loc, table1_full, cfg.hid, "ag1")
        cc2 = None

        def spmm(which, table_full, local_keep, out_cb):
            ccs = cc1 if which == 1 else cc2
            wpq = W // Q  # windows per AG chunk
            call_pos = 0
            ci_call = 0
            for sgi in range(n_sg):
                ts_ = list(range(sgi * cfg.sg, min((sgi + 1) * cfg.sg, T)))
                chunkmap = {}
                for c in [c for c in calls if c["sg"] == sgi]:
                    n_slots = c["n"]
                    gd = gpool.tile(
                        [128, maxcall_ch, RP], BF, tag="gd",
                        name=f"g{which}_{ci_call}",
                    )
                    ci_call += 1
                    win = table_full[
                        c["w"] * cfg.win_rows : (c["w"] + 1) * cfg.win_rows, :
                    ]
                    g = nc.gpsimd.dma_gather(
                        gd[:, : n_slots // 128, :],
                        win,
                        isb_all[:, call_pos // 16 : (call_pos + n_slots) // 16],
                        n_slots,
                        n_slots,
                        RP,
                        single_packet=bool(cfg.single_packet) and (n_slots <= 1024),
                        queue_num=qn[0] % 4,
                    )
                    qn[0] += 1
                    add_dep_helper(
                        g.ins, ccs[c["w"] // wpq].ins, reason="gather after allgather"
                    )
                    add_dep_helper(g.ins, lib_inst.ins, reason="gather after lib")
                    for t_, cj_ in c["chunks"]:
                        chunkmap.setdefault((t_, c["w"]), []).append((gd, cj_))
                    call_pos += n_slots
                for t in ts_:
                    nch_t = int(nch[t].sum())
                    oh = None
                    if nch_t > 0:
                        oh = opool.tile(
                            [128, nch_t, 128], BF, tag="oh", name=f"oh{which}_{t}"
                        )
                        ch0 = int(tile_ch0[t])
                        nc.vector.tensor_tensor(
                            out=oh[:],
                            in0=dlc[:, ch0 : ch0 + nch_t].to_broadcast(
                                [128, nch_t, 128]
                            ),
                            in1=iot[:, :, :].to_broadcast([128, nch_t, 128]),
                            op=mybir.AluOpType.is_equal,
                        )
                    if which == 1:
                        ps = psum1.tile([cfg.hid, 128], F32, tag="ps1", name=f"s{t}")
                        # self-loop: ps[f, d] += h1full[d, f] (identity one-hot)
                        nc.tensor.matmul(
                            ps[:], lhsT=local_keep[:, t, :], rhs=idn[:],
                            start=True, stop=(nch_t == 0),
                        )
                    else:
                        ps = psum2.tile(
                            [128, cfg.out_pad], F32, tag="ph", name=f"s2_{t}"
                        )
                        nc.tensor.matmul(
                            ps[:], lhsT=idn[:], rhs=local_keep[:, t, :],
                            start=True, stop=(nch_t == 0),
                        )
                    ci = 0
                    for wi in range(W):
                        if nch[t, wi] == 0:
                            continue
                        for gd, cj in chunkmap[(t, wi)]:
                            gsl = gd[:, cj, : cfg.hid]
                            osl = oh[:, ci, :]
                            if which == 1:
                                nc.tensor.matmul(
                                    ps[:], lhsT=gsl, rhs=osl,
                                    start=False, stop=(ci == nch_t - 1),
                                )
                            else:
                                nc.tensor.matmul(
                                    ps[:], lhsT=osl, rhs=gsl,
                                    start=False, stop=(ci == nch_t - 1),
                                )
                            ci += 1
                    out_cb(t, ps)

        def spmm1_out(t, ps):
            rt = rpool.tile([cfg.hid, 128], BF, tag="rt", name=f"rt{t}")
            nc.scalar.activation(rt[:], ps[:], mybir.ActivationFunctionType.Relu)
            p2 = psum2.tile([128, cfg.out_pad], F32, tag="ph", name=f"p2_{t}")
            nc.tensor.matmul(p2[:], lhsT=rt[:], rhs=w2t[:], start=True, stop=True)
            nc.scalar.activation(
                t2full[:, t, :],
                p2[:],
                mybir.ActivationFunctionType.Copy,
                scale=dv2c[:, t : t + 1],
            )
            sgi, j = t // cfg.sg, t % cfg.sg
            gn = min(cfg.sg, T - sgi * cfg.sg)
            if j == gn - 1:
                q = (sgi * cfg.sg) // tiles_pc
                r0 = (sgi * cfg.sg - q * tiles_pc) * 128
                nc.sync.dma_start(
                    t2loc[q][r0 : r0 + gn * 128, : cfg.out_pad].rearrange(
                        "(j p) f -> p j f", p=128
                    ),
                    t2full[:, sgi * cfg.sg : sgi * cfg.sg + gn, :],
                )

        spmm(1, table1_full, h1full, spmm1_out)

        cc2 = make_ag(t2loc, table2_full, cfg.out_pad, "ag2")

        outgs = {}

        def spmm2_out(t, ps):
            sgi, j = t // cfg.sg, t % cfg.sg
            if sgi not in outgs:
                outgs[sgi] = hpool.tile(
                    [128, cfg.sg, cfg.out_pad], F32, tag="og", name=f"og{sgi}"
                )
            nc.scalar.activation(
                outgs[sgi][:, j, :],
                ps[:],
                mybir.ActivationFunctionType.Copy,
                scale=dvc[:, t : t + 1],
            )
            gn = min(cfg.sg, T - sgi * cfg.sg)
            if j == gn - 1:
                nc.sync.dma_start(
                    out_rows[
                        sgi * cfg.sg * 128 : (sgi * cfg.sg + gn) * 128, :
                    ].rearrange("(j p) f -> p j f", p=128),
                    outgs[sgi][:, :gn, : cfg.out_f],
                )

        spmm(2, table2_full, t2full, spmm2_out)

    nc.compile()
    return nc


class _Runner:
    """Cached PJRT runner (jit built once per compiled kernel)."""

    def __init__(self, nc, n_cores):
        import jax
        import numpy as np
        from jax.sharding import Mesh, PartitionSpec
        from jax.experimental.shard_map import shard_map
        from concourse.bass2jax import (
            _bass_exec_p,
            install_neuronx_cc_hook,
            partition_id_tensor,
        )

        install_neuronx_cc_hook()
        self.n_cores = n_cores
        self._nc = nc
        partition_name = (
            nc.partition_id_tensor.name if nc.partition_id_tensor else None
        )
        in_names, out_names, out_avals, zero_outs = [], [], [], []
        import concourse.mybir as mb

        for alloc in nc.m.functions[0].allocations:
            if not isinstance(alloc, mb.MemoryLocationSet):
                continue
            name = alloc.memorylocations[0].name
            if alloc.kind == "ExternalInput":
                if name != partition_name:
                    in_names.append(name)
            elif alloc.kind == "ExternalOutput":
                out_names.append(name)
                shape = tuple(alloc.tensor_shape)
                dtype = mb.dt.np(alloc.dtype)
                out_avals.append(jax.core.ShapedArray(shape, dtype))
                zero_outs.append(np.zeros(shape, dtype))
        self.in_names = in_names
        self.out_names = out_names
        self.out_avals = out_avals
        self.zero_outs = zero_outs
        n_params = len(in_names)
        self.n_params = n_params
        all_in_names = in_names + out_names
        if partition_name is not None:
            all_in_names.append(partition_name)

        def _body(*args):
            operands = list(args)
            if partition_name is not None:
                operands.append(partition_id_tensor())
            outs = _bass_exec_p.bind(
                *operands,
                out_avals=tuple(out_avals),
                in_names=tuple(all_in_names),
                out_names=tuple(out_names),
                lowering_input_output_aliases=(),
                sim_require_finite=True,
                sim_require_nnan=True,
                nc=nc,
            )
            return tuple(outs)

        devices = jax.devices()[:n_cores]
        assert len(devices) >= n_cores, f"need {n_cores} cores, have {devices}"
        mesh = Mesh(np.asarray(devices[:n_cores]), ("core",))
        in_specs = (PartitionSpec("core"),) * (n_params + len(out_names))
        out_specs = (PartitionSpec("core"),) * len(out_names)
        self.fn = jax.jit(
            shard_map(
                _body,
                mesh=mesh,
                in_specs=in_specs,
                out_specs=out_specs,
                check_rep=False,
            ),
            keep_unused=True,
        )
        self._jax = jax

    def _concat(self, in_maps):
        per_core = [[np.asarray(m[name]) for name in self.in_names] for m in in_maps]
        concat_in = [
            np.concatenate([per_core[c][i] for c in range(self.n_cores)], axis=0)
            for i in range(self.n_params)
        ]
        concat_zeros = [
            np.zeros((self.n_cores * z.shape[0], *z.shape[1:]), z.dtype)
            for z in self.zero_outs
        ]
        return concat_in + concat_zeros

    def stage(self, in_maps):
        self._dev = self._jax.device_put(self._concat(in_maps))
        self._jax.block_until_ready(self._dev)

    def run_staged(self):
        outs = self.fn(*self._dev)
        self._jax.block_until_ready(outs)
        return outs

    def split(self, outs):
        return [
            {
                name: np.asarray(outs[i]).reshape(
                    self.n_cores, *self.out_avals[i].shape
                )[c]
                for i, name in enumerate(self.out_names)
            }
            for c in range(self.n_cores)
        ]

    def run(self, in_maps):
        outs = self.fn(*self._concat(in_maps))
        self._jax.block_until_ready(outs)
        return self.split(outs)


_CACHE = {}


def _get_runner(layout):
    caps_key = hashlib.sha256(np.ascontiguousarray(layout["caps"]).tobytes()).hexdigest()
    if caps_key not in _CACHE:
        nc = _build_kernel(CFG, layout)
        _CACHE[caps_key] = _Runner(nc, CFG.n_cores)
    return _CACHE[caps_key]


def kernel(x, W1, W2, edge_index):
    cfg = CFG
    in_maps, layout = _preprocess(cfg, x, W1, W2, edge_index)
    runner = _get_runner(layout)

    def _once():
        results = runner.run(in_maps)
        parts = [results[c]["out_rows"] for c in range(cfg.n_cores)]
        return np.ascontiguousarray(
            np.concatenate(parts, axis=0)[: cfg.n, :], dtype=np.float32
        )

    out = _once()
    # Flaky axon devices occasionally come up desynced and return garbage on
    # the first execution after a wedge; one cheap re-run is reliable.
    if not np.isfinite(out).all() or float(np.abs(out).sum()) == 0.0:
        out = _once()
    return out


def _internals():
    return CFG, _preprocess, _build_kernel, _Runner
